# revision 28
# baseline (speedup 1.0000x reference)
"""Windowed attention (swin-style, 49-token windows, 8 heads) with DynamicPosBias.

Data-parallel over B=2048 windows -> 256 windows/core on 8 cores. The tiny DPB
MLP runs on host (numpy); its output (the 169x8 bias table) is folded into the
QK matmul as 49 identity contraction rows. Windows are processed two-at-a-time
("pairs"): one matmul per (pair, head) computes both windows' 49x49 logits in a
[115,98]x[115,98] product whose cross-window blocks are pushed to -400 via two
extra indicator contraction rows, so exp() underflows them to exactly 0 in fp16.
PV then contracts over all 98 stacked keys against stacked V; softmax
denominators come from N=1 matmuls against a ones vector. 8 windows per
iteration share one input DMA for q+k, one for v, one output DMA.
"""

import numpy as np
from contextlib import ExitStack

import concourse.bass as bass
import concourse.mybir as mybir
import concourse.tile as tile
from concourse import bacc
from concourse.bass_utils import run_bass_kernel_spmd

G = 7
NTOK = 49          # tokens per window
H = 8              # heads
HD = 64            # head dim
C = 512
B = 2048
NCORES = 8
W = B // NCORES    # windows per core (256)
WPI = 8            # windows per iteration
ITERS = W // WPI   # 32
NPAIR = W // 2     # 128 window pairs per core
PDIM = 32          # MLP hidden
NBUF = 4           # v slot buffers
NQK = 3            # kt/qt slot buffers
KROWS = 115        # 64 head dims + 49 identity rows + 2 window-indicator rows
TCOLS = 3136       # per-tile cols (g=4, h=8, b=2, x=49)
NEG = -400.0       # cross-window mask: exp(0.125 * -400) underflows fp16 to 0
F32 = mybir.dt.float32
F16 = mybir.dt.float16
F8 = mybir.dt.float8e3  # e3m4: k-side quantization, rel err ~1.3e-2 end to end


def _rel_idx():
    coords = np.stack(np.meshgrid(np.arange(G), np.arange(G), indexing="ij")).reshape(2, -1)
    rel = (coords[:, :, None] - coords[:, None, :]).transpose(1, 2, 0).copy()
    rel[:, :, 0] += G - 1
    rel[:, :, 1] += G - 1
    rel[:, :, 0] *= 2 * G - 1
    return rel.sum(-1)  # [query i, key j] in [0, 169)


def _biases():
    pb = np.arange(1 - G, G, dtype=np.float32)
    return np.stack(np.meshgrid(pb, pb, indexing="ij")).reshape(2, -1).T  # [169, 2]


def _ln(x, g, b, eps=1e-5):
    mu = x.mean(-1, keepdims=True)
    var = ((x - mu) ** 2).mean(-1, keepdims=True)
    return (x - mu) / np.sqrt(var + eps) * g + b


def _pos_table(pos_proj_w, pos_proj_b, ln1_g, ln1_b, w1, b1,
               ln2_g, ln2_b, w2, b2, ln3_g, ln3_b, w3, b3):
    x = _biases() @ pos_proj_w + pos_proj_b
    x = np.maximum(_ln(x, ln1_g, ln1_b), 0) @ w1 + b1
    x = np.maximum(_ln(x, ln2_g, ln2_b), 0) @ w2 + b2
    x = np.maximum(_ln(x, ln3_g, ln3_b), 0) @ w3 + b3
    return x.astype(np.float32)  # [169, H]


def _const_rows(pos):
    """Rows 64:115 of the kt (lhsT, fp8e3) and qt (rhs, fp16) slots.

    Col layout (g, h, b, x). Row 64+j, j<49: kt = I[j,x] (exact in e3m4),
    qt = 8*pos[REL_IDX[x,j],h]. Rows 113/114 are window indicators on kt and
    NEG masks on qt, adding NEG to the (b=0,b'=1)/(b=1,b'=0) cross blocks.
    """
    import ml_dtypes
    ridx = _rel_idx()  # [query, key]
    rpb8 = 8.0 * pos[ridx]                       # [query x, key j, h]
    ck = np.zeros((51, 4, H, 2, NTOK), np.float32)
    cq = np.zeros((51, 4, H, 2, NTOK), np.float32)
    eye = np.eye(NTOK, dtype=np.float32)
    ck[0:49] = eye[:, None, None, None, :]
    cq[0:49] = rpb8.transpose(1, 2, 0)[:, None, :, None, :]
    ck[49, :, :, 0, :] = 1.0   # lhsT row 113: indicator of window b=0
    ck[50, :, :, 1, :] = 1.0   # lhsT row 114: indicator of window b=1
    cq[49, :, :, 1, :] = NEG   # rhs row 113: NEG on b'=1 columns
    cq[50, :, :, 0, :] = NEG   # rhs row 114: NEG on b'=0 columns
    ck8 = np.ascontiguousarray(ck.reshape(51, TCOLS).astype(ml_dtypes.float8_e3m4))
    cq16 = np.ascontiguousarray(cq.reshape(51, TCOLS).astype(np.float16))
    return ck8, cq16


_CACHED_NC = None
LAST_RESULTS = None


def _build_nc():
    global _CACHED_NC
    if _CACHED_NC is not None:
        return _CACHED_NC
    nc = bacc.Bacc(None, target_bir_lowering=False)

    VW = 4 * H * 65  # v slot cols: (g, h, c65) with fused ones column
    # rows 64:115 of kt/qt carry the identity/rpb/mask consts, repeated per
    # iter: the DMA pipe charges per-partition bytes, so the extra rows ride
    # along free and remove any separate const transfer
    kt_d = nc.dram_tensor("ktd", [ITERS, KROWS, TCOLS], F8, kind="ExternalInput")
    qt_d = nc.dram_tensor("qtd", [ITERS, KROWS, TCOLS], F16, kind="ExternalInput")
    v_d = nc.dram_tensor("v", [ITERS, 2 * NTOK, VW], F16, kind="ExternalInput")
    out_d = nc.dram_tensor("out", [ITERS, 2 * NTOK, 4 * C], F16, kind="ExternalOutput")

    EXPF = mybir.ActivationFunctionType.Exp
    MULT = mybir.AluOpType.mult

    with tile.TileContext(nc) as tc, ExitStack() as ctx:
        const = ctx.enter_context(tc.tile_pool(name="const", bufs=1))
        kt_slots = [const.tile([KROWS, TCOLS], F8, tag=f"kt{s}", name=f"kt{s}") for s in range(NQK)]
        qt_slots = [const.tile([KROWS, TCOLS], F16, tag=f"qt{s}", name=f"qt{s}") for s in range(NQK)]
        v_slots = [const.tile([2 * NTOK, VW], F16, tag=f"v{s}", name=f"v{s}") for s in range(NBUF)]

        stp = ctx.enter_context(tc.tile_pool(name="stp", bufs=2, space="PSUM"))
        pvp = ctx.enter_context(tc.tile_pool(name="pvp", bufs=2, space="PSUM"))
        exq = ctx.enter_context(tc.tile_pool(name="exq", bufs=4))
        recp = ctx.enter_context(tc.tile_pool(name="recp", bufs=2))
        outp = ctx.enter_context(tc.tile_pool(name="outp", bufs=3))

        # startup: iter 0 then iter 1
        for s in range(2):
            nc.sync.dma_start(kt_slots[s][:, :], kt_d[s])
            nc.sync.dma_start(qt_slots[s][:, :], qt_d[s])
            nc.sync.dma_start(v_slots[s][:, :], v_d[s])

        DEPTH = 3
        inflight = {}
        out_holder = {}
        for p in range(NPAIR + DEPTH):
            it, g = divmod(p, 4)
            if p < NPAIR:
                nxt = it + 2
                if g == 0 and nxt < ITERS:
                    nc.sync.dma_start(kt_slots[nxt % NQK][:, :], kt_d[nxt])
                    nc.sync.dma_start(qt_slots[nxt % NQK][:, :], qt_d[nxt])
                    nc.sync.dma_start(v_slots[nxt % NBUF][:, :], v_d[nxt])
                s = it % NQK
                sv = it % NBUF
                stA = stp.tile([98, 392], F32, tag="stA")
                stB = stp.tile([98, 392], F32, tag="stB")
                for h in range(H):
                    st = stA if h < 4 else stB
                    j = h % 4
                    base = (g * H + h) * 98
                    nc.tensor.matmul(
                        out=st[:, 98 * j : 98 * j + 98],
                        lhsT=kt_slots[s][0:KROWS, base : base + 98],
                        rhs=qt_slots[s][0:KROWS, base : base + 98],
                        start=True, stop=True,
                    )
                exA = exq.tile([98, 392], F16, tag="exA")
                exB = exq.tile([98, 392], F16, tag="exB")
                nc.scalar.activation(exA[:], stA[:], EXPF, scale=0.125)
                nc.scalar.activation(exB[:], stB[:], EXPF, scale=0.125)
                inflight[p] = (exA, exB, sv, g, it)
            if p >= DEPTH:
                exA, exB, s2, g2, it2 = inflight.pop(p - DEPTH)
                pvA = pvp.tile([98, 260], F32, tag="pvA")
                pvB = pvp.tile([98, 260], F32, tag="pvB")
                for h in range(H):
                    ex = exA if h < 4 else exB
                    pv = pvA if h < 4 else pvB
                    j = h % 4
                    nc.tensor.matmul(
                        out=pv[:, 65 * j : 65 * j + 65],
                        lhsT=ex[:, 98 * j : 98 * j + 98],
                        rhs=v_slots[s2][:, 520 * g2 + 65 * h : 520 * g2 + 65 * h + 65],
                        start=True, stop=True,
                    )
                if g2 == 0:
                    out_t = outp.tile([98, 4 * C], F16, tag="out", name=f"o{it2 % 3}")
                    out_holder[it2] = out_t
                out_t = out_holder[it2]
                for t, pv in ((0, pvA), (1, pvB)):
                    rec = recp.tile([98, 4], F32, tag=f"rec{t}", name=f"rec{t}")
                    nc.vector.reciprocal(
                        rec[:].rearrange("p (h o) -> p h o", o=1),
                        pv[:].rearrange("p (h c) -> p h c", c=65)[:, :, 64:65],
                    )
                    nc.vector.tensor_tensor(
                        out=out_t[:, C * g2 + 256 * t : C * g2 + 256 * t + 256]
                            .rearrange("p (h c) -> p h c", c=HD),
                        in0=pv[:].rearrange("p (h c) -> p h c", c=65)[:, :, 0:HD],
                        in1=rec[:].rearrange("p (h o) -> p h o", o=1).to_broadcast([98, 4, HD]),
                        op=MULT,
                    )
                if g2 == 3:
                    # merged leading dims on the DRAM dst AP: identical 1KB
                    # descriptors, 4x cheaper in the DMA pipe
                    nc.gpsimd.dma_start(
                        out_d[it2].rearrange("p (g c) -> (p g) c", c=C),
                        out_holder.pop(it2)[:].rearrange("p (g c) -> p g c", c=C),
                    )

    nc.finalize()
    _CACHED_NC = nc
    return nc


def kernel(q, k, v, pos_proj_w, pos_proj_b, ln1_g, ln1_b, w1, b1,
           ln2_g, ln2_b, w2, b2, ln3_g, ln3_b, w3, b3):
    q = np.asarray(q, dtype=np.float32)
    k = np.asarray(k, dtype=np.float32)
    v = np.asarray(v, dtype=np.float32)

    pos = _pos_table(
        np.asarray(pos_proj_w, np.float32), np.asarray(pos_proj_b, np.float32),
        np.asarray(ln1_g, np.float32), np.asarray(ln1_b, np.float32),
        np.asarray(w1, np.float32), np.asarray(b1, np.float32),
        np.asarray(ln2_g, np.float32), np.asarray(ln2_b, np.float32),
        np.asarray(w2, np.float32), np.asarray(b2, np.float32),
        np.asarray(ln3_g, np.float32), np.asarray(ln3_b, np.float32),
        np.asarray(w3, np.float32), np.asarray(b3, np.float32),
    )
    ck8, cq16 = _const_rows(pos)
    import ml_dtypes

    in_maps = []
    for c in range(NCORES):
        sl = slice(c * W, (c + 1) * W)
        # kt/qt: [iters, 115, (g, h, b, x)]; rows 64:115 = consts, k in fp8 e3m4
        qc = q[sl].reshape(ITERS, 4, 2, NTOK, H, HD).transpose(0, 5, 1, 4, 2, 3)
        kc = k[sl].reshape(ITERS, 4, 2, NTOK, H, HD).transpose(0, 5, 1, 4, 2, 3)
        kt = np.concatenate(
            [kc.reshape(ITERS, HD, TCOLS).astype(ml_dtypes.float8_e3m4),
             np.broadcast_to(ck8, (ITERS, 51, TCOLS))], axis=1)
        qt = np.concatenate(
            [qc.reshape(ITERS, HD, TCOLS).astype(np.float16),
             np.broadcast_to(cq16, (ITERS, 51, TCOLS))], axis=1)
        # v: [iters, (b, k), (g, h, c65)] with ones column per head (denominator)
        vc = v[sl].reshape(W, NTOK, H, HD)
        vc = np.concatenate([vc, np.ones((W, NTOK, H, 1), np.float32)], axis=3)
        vc = vc.reshape(ITERS, 4, 2, NTOK, H * 65).transpose(0, 2, 3, 1, 4)
        vc = vc.reshape(ITERS, 2 * NTOK, 4 * H * 65).astype(np.float16)
        in_maps.append({
            "ktd": np.ascontiguousarray(kt),
            "qtd": np.ascontiguousarray(qt),
            "v": np.ascontiguousarray(vc),
        })

    nc = _build_nc()
    res = run_bass_kernel_spmd(nc, in_maps, core_ids=list(range(NCORES)))
    global LAST_RESULTS
    LAST_RESULTS = res
    # out: [iters, (b, q), (g, c)] -> [W, 49, 512]
    parts = []
    for r in res.results:
        o = r["out"].reshape(ITERS, 2, NTOK, 4, C).transpose(0, 3, 1, 2, 4)
        parts.append(o.reshape(W, NTOK, C))
    return np.concatenate(parts, axis=0).astype(np.float32)
